# revision 8
# baseline (speedup 1.0000x reference)
"""Trainium2 Bass kernel for nn_ClusteredAttention_26001732010424.

Math (see reference):
    sum_tot_vec = key.sum(axis=2)                          # (b, l, s) pooled key
    scores[b,l,v,m] = <query[b,l,v,:], sum_tot_vec[b,m,:]>
    A = softmax(scale * scores, axis=-1)                   # over m
    V[b,l,v,s] = sum_m A[b,l,v,m] * value[b,m,v,s]

Sharding: the 16 (b, v) pairs are independent given the pooled key, so core i
handles head v=i for both batches (2 pairs/core, 8 cores). The tiny pooled-key
reduction (0.4% of FLOPs) is done host-side and broadcast, so no collectives.

Device pipeline per core (qt carries log2e/8 so both exp engines compute 2^t):
    S^T[m, l] = kt[s, m]^T-matmul qt[s, l]   (contraction s=64, fp32r, PSUM)
    es = 2^S^T in bf16, split across TWO engines: ScalarE Exp(scale=ln2) and
         GPSIMD pow(2, S^T) (the Pool engine's software ALU implements pow),
         statically load-balanced so both finish together.
    U[l, s+1] += es[m, l]^T-matmul va[m, s+1]  accumulated over m-tiles in
         PSUM. va is bf16 with a ones column, so column s holds the softmax
         denominator; the division happens on host. The flipped orientation
         (l on partitions, s+1 moving) makes each AV matmul cost 65 moving
         rows instead of 512.
"""

import os

import numpy as np

# NTFF trace hooks (antenv.axon_hooks) are not present in all runtime
# environments; tracing is never needed for correctness, so hard-disable it.
os.environ["BASS_NEVER_TRACE"] = "1"

import ml_dtypes

import concourse.bacc as bacc
import concourse.mybir as mybir
import concourse.tile as tile
from concourse.bass_utils import run_bass_kernel_spmd

B, L, V, S = 2, 2048, 8, 64
P = 128  # partitions / m-tile rows
MT = L // P  # m-tiles per pair (16)
F32 = mybir.dt.float32
F32R = mybir.dt.float32r
BF16 = mybir.dt.bfloat16
LOG2E = float(np.log2(np.e))
LN2 = float(np.log(2.0))

_CACHED_NC = None

# Per-chunk exp cost estimates from the TRN2 cost model, used for the static
# ACT/Pool load balance: ACT 0.8333 ns/col + 185 ns/instr, Pool (GPSIMD
# software pow) 1.389 ns/col + 95 ns/instr.
def _assign_exp_engines(chunks):
    busy = {"act": 0.0, "pool": 0.0}
    out = []
    n = len(chunks)
    for ci, chunk in enumerate(chunks):
        cols = len(chunk) * 512
        t_act = cols * (1 / 1.2) + 185.0
        t_pool = cols * (1 / 1.2) / 0.6 + 95.0
        if ci >= n - 2:
            eng = "act"  # short tail: last exps on the faster engine
        elif ci == 0:
            eng = "act"
        elif ci == 1:
            eng = "pool"
        else:
            eng = "act" if busy["act"] + t_act <= busy["pool"] + t_pool else "pool"
        busy[eng] += t_act if eng == "act" else t_pool
        out.append(eng)
    return out


def _build_nc():
    nc = bacc.Bacc("TRN2", target_bir_lowering=False, debug=False, num_devices=8)

    qt = nc.dram_tensor("qt", (B, S, L), F32R, kind="ExternalInput")
    kt = nc.dram_tensor("kt", (B, S, L), F32R, kind="ExternalInput")
    va = nc.dram_tensor("va", (B, P, MT, S + 1), BF16, kind="ExternalInput")
    # out[b, h, j, l_part, sub, s]: l = h*1024 + j*512 + sub*128 + l_part
    out = nc.dram_tensor("out", (B, 2, 2, P, 4, S + 1), F32, kind="ExternalOutput")

    with tile.TileContext(nc) as tc:
        with (
            tc.tile_pool(name="inp", bufs=2) as inp,
            tc.tile_pool(name="es", bufs=6) as esp,
            tc.tile_pool(name="ts", bufs=2) as tsp,
            tc.tile_pool(name="outp", bufs=4) as outp,
            tc.tile_pool(name="wz", bufs=1) as wzp,
            tc.tile_pool(name="st", bufs=2, space="PSUM") as stp,
            tc.tile_pool(name="up", bufs=2, space="PSUM") as upp,
        ):
            # PE warmup: dummy matmuls on zeros during the DMA fill keep the
            # PE ramp (HAM) warm so real matmuls start at full clock. Output
            # goes to an st-pool slot; the first real scores overwrite it.
            zsrc = wzp.tile([S, 512], BF16)
            nc.vector.memset(zsrc[:], 0.0)
            # base-2 operand for the GPSIMD pow path (pow(2, t) == 2^t)
            two = wzp.tile([P, 1536], F32)
            nc.vector.memset(two[:], 2.0)
            warm = stp.tile([P, 1536], F32, tag="st")
            for _ in range(6):
                nc.tensor.matmul(
                    warm[0:S, 0:512],
                    lhsT=zsrc[:, 0:S],
                    rhs=zsrc[:],
                    start=True,
                    stop=True,
                )

            # Input prefetch for BOTH pairs up front, first-needed data first
            # (kt m-tile 0 and qt l-cols 0:512 feed the first score tile).
            qt_sbs, kt_sbs, va_sbs = [], [], []
            for b in range(B):
                qt_sb = inp.tile([S, L], F32R, tag="qt")
                kt_sb = inp.tile([S, L], F32R, tag="kt")
                va_sb = inp.tile([P, MT, S + 1], BF16, tag="va")
                nc.sync.dma_start(kt_sb[:, 0:128], kt.ap()[b, :, 0:128])
                nc.sync.dma_start(qt_sb[:, 0:512], qt.ap()[b, :, 0:512])
                nc.sync.dma_start(kt_sb[:, 128:1024], kt.ap()[b, :, 128:1024])
                nc.sync.dma_start(qt_sb[:, 512:2048], qt.ap()[b, :, 512:2048])
                nc.sync.dma_start(va_sb[:, 0:4], va.ap()[b, :, 0:4])
                nc.sync.dma_start(kt_sb[:, 1024:2048], kt.ap()[b, :, 1024:2048])
                nc.sync.dma_start(va_sb[:, 4:16], va.ap()[b, :, 4:16])
                qt_sbs.append(qt_sb)
                kt_sbs.append(kt_sb)
                va_sbs.append(va_sb)

            # One global stream of 512-col units over (section, t, j) where a
            # section is a (b, h) pair owning l-half h. Chunks of 3 units
            # (1536 cols) share one exp instruction; chunks never span
            # sections (per-section pattern 10x3 + 1x2) so the section's PSUM
            # accumulators turn over cleanly.
            sections = [(b, h) for b in range(B) for h in range(2)]
            chunks = []
            for b, h in sections:
                units = [(b, h, t, j) for t in range(MT) for j in range(2)]
                chunks.extend(units[i : i + 3] for i in range(0, 30, 3))
                chunks.append(units[30:])
            engines = _assign_exp_engines(chunks)

            # Softmax accumulators: per section, two 1-bank tiles (j = 0, 1),
            # each holding 4 l-blocks of 128 at a 512B stride so every matmul
            # output stays bank-aligned. The pool's 2 slots give j=0/j=1 of
            # consecutive sections natural single-buffered reuse.
            u_tiles = {}

            def get_u(b, h, j):
                if (b, h, j) not in u_tiles:
                    u_tiles[(b, h, j)] = upp.tile(
                        [P, 4, P], F32, tag="u", name=f"u_{b}_{h}_{j}"
                    )
                return u_tiles[(b, h, j)]

            def issue_av(chunk, es):
                for i, (b, h, t, j) in enumerate(chunk):
                    u = get_u(b, h, j)
                    for sub in range(4):
                        # start_tensor_calc zeroes the WHOLE PSUM bank, so
                        # only the very first matmul into this bank may set
                        # it; later sub-blocks accumulate onto the zeroed
                        # region (skip_group_check: one group per bank with
                        # interleaved start/stop flags confuses the checker).
                        nc.tensor.matmul(
                            u[:, sub, 0 : S + 1],
                            lhsT=es[:, i * 512 + sub * 128 : i * 512 + (sub + 1) * 128],
                            rhs=va_sbs[b][:, t, :],
                            start=(t == 0 and sub == 0),
                            stop=(t == MT - 1 and sub == 3),
                            skip_group_check=True,
                        )
                    if t == MT - 1:
                        # accumulator (b, h, j) complete: evacuate and store
                        out_sb = outp.tile([P, 4, S + 1], F32, tag="out")
                        nc.vector.tensor_copy(out_sb[:], u[:, :, 0 : S + 1])
                        nc.sync.dma_start(out.ap()[b, h, j], out_sb[:])
                        del u_tiles[(b, h, j)]

            # AV trails scores/exp by two chunks: placed after the NEXT
            # chunk's scores in program order, so the PE works on the scores
            # that feed the bottleneck exp engines first, and a new section's
            # first AV (waiting on the previous accumulator's evacuation)
            # never starves them.
            pending = []
            for ci, chunk in enumerate(chunks):
                n = len(chunk)
                st = stp.tile([P, n * 512], F32, tag="st")
                for i, (b, h, t, j) in enumerate(chunk):
                    l0 = h * 1024 + j * 512
                    nc.tensor.matmul(
                        st[:, i * 512 : (i + 1) * 512],
                        lhsT=kt_sbs[b][:, t * P : (t + 1) * P],
                        rhs=qt_sbs[b][:, l0 : l0 + 512],
                        start=True,
                        stop=True,
                    )
                es = esp.tile([P, n * 512], BF16, tag="es")
                if engines[ci] == "act":
                    nc.scalar.activation(
                        es[:], st[:], mybir.ActivationFunctionType.Exp, scale=LN2
                    )
                else:
                    # GPSIMD cannot read PSUM; DVE stages the scores to SBUF
                    ts = tsp.tile([P, n * 512], F32, tag="ts")
                    nc.vector.tensor_copy(ts[:], st[:])
                    nc.gpsimd.tensor_tensor(
                        es[:], two[:, 0 : n * 512], ts[:], mybir.AluOpType.pow
                    )
                pending.append((chunk, es))
                if len(pending) > 2:
                    issue_av(*pending.pop(0))
            for p in pending:
                issue_av(*p)

    nc.compile()
    return nc


def kernel(query, key, value, label_arr=None, **_unused):
    global _CACHED_NC
    query = np.asarray(query, dtype=np.float32)
    key = np.asarray(key, dtype=np.float32)
    value = np.asarray(value, dtype=np.float32)

    # Fold the softmax scale and the base-2 conversion into q so the device
    # computes 2^(qt.kt) on both exp engines.
    scale = np.float32(LOG2E / np.sqrt(S))

    # qt[b, v, s, l] = query[b, l, v, s] * scale
    qt = np.ascontiguousarray(
        np.transpose(query * scale, (0, 2, 3, 1))
    )  # (B, V, S, L)

    # kt[b, s, m] = sum_v key[b, m, v, s]
    kt = np.ascontiguousarray(np.transpose(key.sum(axis=2), (0, 2, 1)))

    # va[b, v, p, t, c]: value with a ones column, partition-major, bf16:
    # va[b, v, p, t, :S] = value[b, t*128+p, v, :], va[..., S] = 1
    va = np.ones((B, L, V, S + 1), dtype=np.float32)
    va[:, :, :, :S] = value
    # (b, l, v, c) -> (b, t, p, v, c) -> (b, v, p, t, c)
    va = (
        va.reshape(B, MT, P, V, S + 1)
        .transpose(0, 3, 2, 1, 4)
        .astype(ml_dtypes.bfloat16)
    )
    va = np.ascontiguousarray(va)

    if _CACHED_NC is None:
        _CACHED_NC = _build_nc()
    nc = _CACHED_NC

    in_maps = [
        {
            "qt": np.ascontiguousarray(qt[:, v]),
            "kt": kt,
            "va": np.ascontiguousarray(va[:, v]),
        }
        for v in range(V)
    ]
    res = run_bass_kernel_spmd(nc, in_maps, core_ids=list(range(8)))
    global _LAST_EXEC_NS
    _LAST_EXEC_NS = res.exec_time_ns

    result = np.empty((B, L, V, S), dtype=np.float32)
    for v in range(V):
        o = res.results[v]["out"]  # (B, 2, 2, P, 4, S+1)
        vt = o[..., :S] / o[..., S : S + 1]  # (B, 2, 2, P, 4, S)
        # l = h*1024 + j*512 + sub*128 + l_part -> axes (h, j, sub, l_part)
        result[:, :, v, :] = vt.transpose(0, 1, 2, 4, 3, 5).reshape(B, L, S)
    return result


# revision 12
# speedup vs baseline: 1.0321x; 1.0321x over previous
"""Trainium2 Bass kernel for nn_ClusteredAttention_26001732010424.

Math (see reference):
    sum_tot_vec = key.sum(axis=2)                          # (b, l, s) pooled key
    scores[b,l,v,m] = <query[b,l,v,:], sum_tot_vec[b,m,:]>
    A = softmax(scale * scores, axis=-1)                   # over m
    V[b,l,v,s] = sum_m A[b,l,v,m] * value[b,m,v,s]

Sharding: the 16 (b, v) pairs are independent given the pooled key, so core i
handles head v=i for both batches (2 pairs/core, 8 cores). The tiny pooled-key
reduction (0.4% of FLOPs) is done host-side and broadcast, so no collectives.

Device pipeline per core (qt carries log2e/8 so both exp engines compute 2^t):
    S^T[m, l] = kt[s, m]^T-matmul qt[s, l]   (contraction s=64, fp32r, PSUM)
    es = 2^S^T in bf16, split across TWO engines: ScalarE Exp(scale=ln2) and
         GPSIMD pow(2, S^T) (the Pool engine's software ALU implements pow;
         it cannot read PSUM, so the otherwise-idle DVE stages those chunks
         to SBUF first), statically load-balanced so both finish together.
    U[l, s+1] += es[m, l]^T-matmul va[m, s+1]  accumulated over m-tiles in
         PSUM. va is bf16 with a ones column, so column s holds the softmax
         denominator; the division happens on host. The flipped orientation
         (l on partitions, s+1 moving) makes each AV matmul cost 65 moving
         rows instead of 512.

The unit stream walks (b, hj-quarter, m-tile): each 512-wide l-quarter's
accumulator sees all 16 m-tiles consecutively, so exactly one PSUM
accumulator bank is filling at any time and they retire staggered — no
end-of-stream PSUM burst, no inter-section stalls.
"""

import os

import numpy as np

# NTFF trace hooks (antenv.axon_hooks) are not present in all runtime
# environments; tracing is never needed for correctness, so hard-disable it.
os.environ["BASS_NEVER_TRACE"] = "1"

import ml_dtypes

import concourse.bacc as bacc
import concourse.mybir as mybir
import concourse.tile as tile
from concourse.bass_utils import run_bass_kernel_spmd

B, L, V, S = 2, 2048, 8, 64
P = 128  # partitions / m-tile rows
MT = L // P  # m-tiles per pair (16)
F32 = mybir.dt.float32
F32R = mybir.dt.float32r
BF16 = mybir.dt.bfloat16
LOG2E = float(np.log2(np.e))
LN2 = float(np.log(2.0))

_CACHED_NC = None


# Per-chunk exp cost estimates from the TRN2 cost model, used for the static
# ACT/Pool load balance: ACT 0.8333 ns/col + 185 ns/instr, Pool (GPSIMD
# software pow) 1.389 ns/col + 95 ns/instr.
def _assign_exp_engines(chunks):
    busy = {"act": 0.0, "pool": 0.0}
    out = []
    n = len(chunks)
    for ci, chunk in enumerate(chunks):
        cols = len(chunk) * 512
        t_act = cols * (1 / 1.2) + 185.0
        t_pool = cols * (1 / 1.2) / 0.6 + 95.0
        if ci >= n - 2 or ci < 3:
            # fast start (DVE is still loading va) and short tail
            eng = "act"
        else:
            eng = "act" if busy["act"] + t_act <= busy["pool"] + t_pool else "pool"
        busy[eng] += t_act if eng == "act" else t_pool
        out.append(eng)
    return out


def _build_nc():
    nc = bacc.Bacc("TRN2", target_bir_lowering=False, debug=False, num_devices=8)

    qt = nc.dram_tensor("qt", (B, S, L), F32R, kind="ExternalInput")
    kt = nc.dram_tensor("kt", (B, S, L), F32R, kind="ExternalInput")
    va = nc.dram_tensor("va", (B, P, MT, S + 1), BF16, kind="ExternalInput")
    # out[b, hj, l_part, sub, s]: l = hj*512 + sub*128 + l_part
    out = nc.dram_tensor("out", (B, 4, P, 4, S + 1), F32, kind="ExternalOutput")

    with tile.TileContext(nc) as tc:
        with (
            tc.tile_pool(name="inp", bufs=2) as inp,
            tc.tile_pool(name="es", bufs=6) as esp,
            tc.tile_pool(name="ts", bufs=2) as tsp,
            tc.tile_pool(name="outp", bufs=4) as outp,
            tc.tile_pool(name="wz", bufs=1) as wzp,
            tc.tile_pool(name="st", bufs=2, space="PSUM") as stp,
            tc.tile_pool(name="up", bufs=2, space="PSUM") as upp,
        ):
            # PE warmup: dummy matmuls on zeros during the DMA fill keep the
            # PE ramp (HAM) warm so real matmuls start at full clock. Output
            # goes to an st-pool slot; the first real scores overwrite it.
            zsrc = wzp.tile([S, 512], BF16)
            nc.vector.memset(zsrc[:], 0.0)
            # base-2 operand for the GPSIMD pow path (pow(2, t) == 2^t)
            two = wzp.tile([P, 1536], F32)
            nc.vector.memset(two[:], 2.0)
            warm = stp.tile([P, 1536], F32, tag="st")
            for _ in range(6):
                nc.tensor.matmul(
                    warm[0:S, 0:512],
                    lhsT=zsrc[:, 0:S],
                    rhs=zsrc[:],
                    start=True,
                    stop=True,
                )

            # Input prefetch, first-needed first. kt/qt stream on the SP
            # queue in consumption order; va rides the DVE queue (idle until
            # the first Pool staging copy) so it never delays the scores.
            qt_sbs, kt_sbs, va_sbs = [], [], []
            for b in range(B):
                qt_sbs.append(inp.tile([S, L], F32R, tag="qt", name=f"qt_sb{b}"))
                kt_sbs.append(inp.tile([S, L], F32R, tag="kt", name=f"kt_sb{b}"))
                va_sbs.append(
                    inp.tile([P, MT, S + 1], BF16, tag="va", name=f"va_sb{b}")
                )
            nc.sync.dma_start(kt_sbs[0][:, 0:512], kt.ap()[0, :, 0:512])
            nc.sync.dma_start(qt_sbs[0][:, 0:512], qt.ap()[0, :, 0:512])
            nc.gpsimd.dma_start(va_sbs[0][:], va.ap()[0])
            nc.sync.dma_start(kt_sbs[0][:, 512:2048], kt.ap()[0, :, 512:2048])
            nc.sync.dma_start(qt_sbs[0][:, 512:2048], qt.ap()[0, :, 512:2048])
            nc.gpsimd.dma_start(va_sbs[1][:], va.ap()[1])
            nc.sync.dma_start(kt_sbs[1][:], kt.ap()[1])
            nc.sync.dma_start(qt_sbs[1][:], qt.ap()[1])

            # One global stream of 512-col units over (b, hj, t): the l-range
            # of unit (b, hj, t) is [hj*512, hj*512+512) and its m-tile is t.
            # Grouped 3 per chunk (1536 cols per exp instruction).
            units = [
                (b, hj, t) for b in range(B) for hj in range(4) for t in range(MT)
            ]
            chunks = [units[i : i + 3] for i in range(0, len(units), 3)]
            engines = _assign_exp_engines(chunks)

            # Softmax accumulators: one 1-bank tile per (b, hj) quarter,
            # holding 4 l-blocks of 128 at a 512B stride so every matmul
            # output stays bank-aligned. The (b, hj, t) stream order means
            # only one is filling at a time; the pool's 2 slots cover
            # fill + evacuate.
            u_tiles = {}

            def get_u(b, hj):
                if (b, hj) not in u_tiles:
                    u_tiles[(b, hj)] = upp.tile(
                        [P, 4, P], F32, tag="u", name=f"u_{b}_{hj}"
                    )
                return u_tiles[(b, hj)]

            def issue_av(chunk, es):
                for i, (b, hj, t) in enumerate(chunk):
                    u = get_u(b, hj)
                    for sub in range(4):
                        # start_tensor_calc zeroes the WHOLE PSUM bank, so
                        # only the very first matmul into this bank may set
                        # it; later sub-blocks accumulate onto the zeroed
                        # region (skip_group_check: one group per bank with
                        # mixed start/stop flags confuses the checker).
                        nc.tensor.matmul(
                            u[:, sub, 0 : S + 1],
                            lhsT=es[:, i * 512 + sub * 128 : i * 512 + (sub + 1) * 128],
                            rhs=va_sbs[b][:, t, :],
                            start=(t == 0 and sub == 0),
                            stop=(t == MT - 1 and sub == 3),
                            skip_group_check=True,
                        )
                    if t == MT - 1:
                        # quarter (b, hj) complete: evacuate PSUM and store
                        out_sb = outp.tile([P, 4, S + 1], F32, tag="out")
                        nc.vector.tensor_copy(out_sb[:], u[:, :, 0 : S + 1])
                        nc.sync.dma_start(out.ap()[b, hj], out_sb[:])
                        del u_tiles[(b, hj)]

            # AV trails scores/exp by two chunks: placed after the NEXT
            # chunk's scores in program order, so the PE works on the scores
            # that feed the bottleneck exp engines first.
            pending = []
            for ci, chunk in enumerate(chunks):
                n = len(chunk)
                st = stp.tile([P, n * 512], F32, tag="st")
                for i, (b, hj, t) in enumerate(chunk):
                    l0 = hj * 512
                    nc.tensor.matmul(
                        st[:, i * 512 : (i + 1) * 512],
                        lhsT=kt_sbs[b][:, t * P : (t + 1) * P],
                        rhs=qt_sbs[b][:, l0 : l0 + 512],
                        start=True,
                        stop=True,
                    )
                es = esp.tile([P, n * 512], BF16, tag="es")
                if engines[ci] == "act":
                    nc.scalar.activation(
                        es[:], st[:], mybir.ActivationFunctionType.Exp, scale=LN2
                    )
                else:
                    # GPSIMD cannot read PSUM; DVE stages the scores to SBUF
                    ts = tsp.tile([P, n * 512], F32, tag="ts")
                    nc.vector.tensor_copy(ts[:], st[:])
                    nc.gpsimd.tensor_tensor(
                        es[:], two[:, 0 : n * 512], ts[:], mybir.AluOpType.pow
                    )
                pending.append((chunk, es))
                if len(pending) > 2:
                    issue_av(*pending.pop(0))
            for p in pending:
                issue_av(*p)

    nc.compile()
    return nc


def kernel(query, key, value, label_arr=None, **_unused):
    global _CACHED_NC
    query = np.asarray(query, dtype=np.float32)
    key = np.asarray(key, dtype=np.float32)
    value = np.asarray(value, dtype=np.float32)

    # Fold the softmax scale and the base-2 conversion into q so the device
    # computes 2^(qt.kt) on both exp engines.
    scale = np.float32(LOG2E / np.sqrt(S))

    # qt[b, v, s, l] = query[b, l, v, s] * scale
    qt = np.ascontiguousarray(
        np.transpose(query * scale, (0, 2, 3, 1))
    )  # (B, V, S, L)

    # kt[b, s, m] = sum_v key[b, m, v, s]
    kt = np.ascontiguousarray(np.transpose(key.sum(axis=2), (0, 2, 1)))

    # va[b, v, p, t, c]: value with a ones column, partition-major, bf16:
    # va[b, v, p, t, :S] = value[b, t*128+p, v, :], va[..., S] = 1
    va = np.ones((B, L, V, S + 1), dtype=np.float32)
    va[:, :, :, :S] = value
    # (b, l, v, c) -> (b, t, p, v, c) -> (b, v, p, t, c)
    va = (
        va.reshape(B, MT, P, V, S + 1)
        .transpose(0, 3, 2, 1, 4)
        .astype(ml_dtypes.bfloat16)
    )
    va = np.ascontiguousarray(va)

    if _CACHED_NC is None:
        _CACHED_NC = _build_nc()
    nc = _CACHED_NC

    in_maps = [
        {
            "qt": np.ascontiguousarray(qt[:, v]),
            "kt": kt,
            "va": np.ascontiguousarray(va[:, v]),
        }
        for v in range(V)
    ]
    res = run_bass_kernel_spmd(nc, in_maps, core_ids=list(range(8)))
    global _LAST_EXEC_NS
    _LAST_EXEC_NS = res.exec_time_ns

    result = np.empty((B, L, V, S), dtype=np.float32)
    for v in range(V):
        o = res.results[v]["out"]  # (B, 4, P, 4, S+1)
        vt = o[..., :S] / o[..., S : S + 1]  # (B, 4, P, 4, S)
        # l = hj*512 + sub*128 + l_part -> axes (hj, sub, l_part)
        result[:, :, v, :] = vt.transpose(0, 1, 3, 2, 4).reshape(B, L, S)
    return result
